# revision 12
# baseline (speedup 1.0000x reference)
"""BigBird block-sparse attention for Trainium2, 8-core SPMD.

Sharding: head-parallel. Each core owns 2 of the 16 heads (both batches).
  - q/k/v projections computed only for the core's 128 feature slice
    (full hidden_states replicated, weights sliced column-wise).
  - attention fully local per (batch, head).
  - out_proj tensor-parallel on the head (contraction) dim: each core
    emits a full-shape partial; the host sums the 8 partials (cheaper
    than a 16MB on-device all-reduce) and adds the output bias.

On-device layout choices (v2):
  - q/k feature-major (features on partitions, tokens on free dim);
    v projected TOKEN-major directly (hT chunk as the stationary matmul
    operand) so the zero-padded v^T slots are filled by plain strided
    copies -- no PE transposes, no transpose phase, PE never idles long
    enough for the HAM clock gate to re-throttle.
  - scores computed transposed: S_T[key, query] = k_j^T q, so that
    * AV is a natural matmul (contraction = keys = partitions),
    * the softmax denominator Z falls out of a ones-column appended to
      V^T,
    * normalization folds into the PSUM->SBUF context copy as a
      partition-broadcast multiply by 1/Z.
  - softmax skips max-subtraction (scores are O(1) after the 1/8 scale;
    exp cannot overflow fp32 for this distribution; softmax is shift
    invariant so the reference is matched).
  - BigBird mask is data independent and block-constant (64x64): it is
    evaluated at trace time into run-lists of attending query blocks per
    128-wide key tile.  No mask tensors on device at all.
  - NO filler matmuls: exp reads stale PSUM in the unwritten half-rows
    of half pieces; those E values are garbage but AV only ever reads
    the real seg ranges (the zero-padded v slots never touch them).
  - softmax normalization (1/Z) is finalized per 1024-column ctx half
    as soon as that half's AV matmuls are issued, so the reciprocal
    chain (copy -> DMA-spread -> recip -> DMA -> broadcast -> mult)
    hides under the other half / the next pair's score matmuls.
"""

import numpy as np
import ml_dtypes
from contextlib import ExitStack

# ----- problem constants (hardcoded per contract) --------------------------
EMBED_DIM = 1024
NUM_HEADS = 16
HEAD_DIM = 64           # d per head
WINDOW = 3
N_RAND = 3
BLOCK = 64
BATCH = 2
SEQ = 2048
NB = SEQ // BLOCK       # 32 key/query blocks per sequence
N_CORES = 8
HPC = NUM_HEADS // N_CORES      # heads per core = 2
FPC = HPC * HEAD_DIM            # feature slice per core = 128
T = BATCH * SEQ                 # 4096 tokens
NKT = NB // 2                   # 16 key tiles of 128 keys per (b,h)
SCALE = HEAD_DIM ** -0.5

BF16 = ml_dtypes.bfloat16

# score-chunk window width in psum columns (2 PSUM banks)
CHUNK_W = 1024
PSUM_BANK = 512  # fp32 elements per bank


def _block_attend() -> np.ndarray:
    """attend[r, kb]: query block r attends key block kb.

    Block-granular replica of the reference _bigbird_mask (the mask is
    block-constant: global first block rows/cols, +-WINDOW band, and
    N_RAND random blocks per row drawn with RandomState(0))."""
    att = np.zeros((NB, NB), dtype=bool)
    att[0, :] = True
    att[:, 0] = True
    blk = np.arange(NB)
    att |= np.abs(blk[:, None] - blk[None, :]) <= WINDOW
    rng = np.random.RandomState(0)
    for b in range(1, NB):
        avail = [x for x in range(1, NB) if abs(x - b) > WINDOW]
        if avail:
            sel = rng.choice(avail, size=min(N_RAND, len(avail)), replace=False)
            att[b, sel] = True
    return att


def _runs_of(mask_1d: np.ndarray):
    """[(r0, nblocks)] maximal runs of consecutive True entries."""
    runs = []
    for r in np.flatnonzero(mask_1d):
        if runs and runs[-1][0] + runs[-1][1] == r:
            runs[-1][1] += 1
        else:
            runs.append([int(r), 1])
    return [(r0, n) for r0, n in runs]


def build_schedule():
    """Global score-column layout + chunking + matmul pairing.

    Scores use ONLY full-tile (128-key) stationary operands over the
    UNION of attending query blocks per key tile: a matmul's cost is its
    streamed column count regardless of stationary width, and the
    zero-padded v slots already discard the non-attending half during
    AV, so splitting scores per 64-key half is pure overhead.

    The per-tile union layouts are concatenated into one global column
    space and cut into CHUNK_W psum windows (not tile-aligned).  AV
    pieces are the per-class (both / kb-even only / kb-odd only) column
    runs, split at chunk and ctx-quarter boundaries.

    Equal-width piece pairs (same key tile / class / psum bank) are
    merged into single matmuls via strided 3-D access patterns, halving
    the fixed per-matmul cost (~170ns SBUF latency + dispatch).

    Returns (chunks, av_q) where
      chunks: [{W, sc: [(j, [(r0, w, off)])], av ignored}]
      and each chunk's av mms, grouped by ctx quarter, are in
      chunk["avq"][quarter] = [(j, cls, [(out_o, w, e_off)])].
    """
    att = _block_attend()
    sc_pieces = []   # (j, r0, nblk, goff)
    av_pieces = []   # (j, cls, r0, nblk, goff)
    goff = 0
    for j in range(NKT):
        a0, a1 = att[:, 2 * j], att[:, 2 * j + 1]
        u = a0 | a1
        cls_arr = np.where(a0 & a1, 2, np.where(a0, 0, 1))
        for r0, n in _runs_of(u):
            sc_pieces.append((j, r0, n, goff))
            i = r0
            while i < r0 + n:
                c = int(cls_arr[i])
                k = i
                while k < r0 + n and cls_arr[k] == c:
                    k += 1
                av_pieces.append((j, c, i, k - i, goff + 64 * (i - r0)))
                i = k
            goff += 64 * n
    W_TOT = goff

    def pair_up(raw, keyfn):
        """Greedy pairing of equal-width pieces. raw items end with a
        sort position; returns list of (head..., [piece, piece?])."""
        from collections import defaultdict
        groups = defaultdict(list)
        order = []
        for it in raw:
            kk = keyfn(it)
            if kk not in groups:
                order.append(kk)
            groups[kk].append(it)
        out = []
        for kk in order:
            g = groups[kk]
            i = 0
            while i < len(g):
                if i + 1 < len(g):
                    out.append([g[i], g[i + 1]])
                    i += 2
                else:
                    out.append([g[i]])
                    i += 1
        return out

    chunks = []
    n_chunks = -(-W_TOT // CHUNK_W)
    for wi in range(n_chunks):
        lo, hi = wi * CHUNK_W, min(W_TOT, (wi + 1) * CHUNK_W)
        W = hi - lo
        # ---- scores: clip, split at S psum banks, pair ----
        sc_raw = []
        for j, r0, n, go in sc_pieces:
            s, e = go, go + 64 * n
            cs, ce = max(s, lo), min(e, hi)
            if cs >= ce:
                continue
            for o, w in _bank_split(cs - lo, ce - cs):
                r = r0 + (lo + o - s) // 64
                sc_raw.append((j, r, w, o))
        sc_pairs = pair_up(sc_raw,
                           lambda t: (t[0], t[2], t[3] // PSUM_BANK))
        sc_mms = []
        for pp in sc_pairs:
            j = pp[0][0]
            pieces = sorted([(r, w, o) for (_j, r, w, o) in pp],
                            key=lambda x: x[2])
            sc_mms.append((j, pieces))
        # ---- AV: clip, split at ctx quarters, pair, group by quarter --
        av_raw = []
        for j, c, r0, n, go in av_pieces:
            s, e = go, go + 64 * n
            cs, ce = max(s, lo), min(e, hi)
            if cs >= ce:
                continue
            r_lo = r0 + (cs - s) // 64
            for o, w in _bank_split(64 * r_lo, ce - cs):
                e_off = (cs - lo) + (o - 64 * r_lo)
                av_raw.append((j, c, o, w, e_off))
        avq = [[] for _ in range(4)]
        av_pairs = pair_up(av_raw,
                           lambda t: (t[0], t[1], t[3], t[2] // PSUM_BANK))
        for pp in av_pairs:
            j, c = pp[0][0], pp[0][1]
            pieces = sorted([(o, w, eo) for (_j, _c, o, w, eo) in pp],
                            key=lambda x: x[0])
            avq[pieces[0][0] // PSUM_BANK].append((j, c, pieces))
        chunks.append(dict(W=W, sc=sc_mms, avq=avq,
                           sc_raw=sc_raw, av_raw=av_raw))
    return chunks


def _bank_split(off, w, bank=PSUM_BANK):
    """split [off, off+w) at bank boundaries -> [(off, w), ...]"""
    out = []
    while w > 0:
        room = bank - (off % bank)
        take = min(room, w)
        out.append((off, take))
        off += take
        w -= take
    return out


# ---------------------------------------------------------------------------
# numpy golden of the exact on-device algorithm (fp32, validates schedule)
# ---------------------------------------------------------------------------
def numpy_golden(hidden_states, wq, bq, wk, bk, wv, bv, wo, bo):
    hs = np.asarray(hidden_states, np.float32).reshape(T, EMBED_DIM)
    chunks = build_schedule()
    out = np.zeros((T, EMBED_DIM), np.float32)
    for c in range(N_CORES):
        f = slice(FPC * c, FPC * (c + 1))
        q = hs @ np.asarray(wq, np.float32)[f, :].T  # (T, 128)
        k = hs @ np.asarray(wk, np.float32)[f, :].T
        v = hs @ np.asarray(wv, np.float32)[f, :].T
        ctx_all = np.zeros((FPC, T), np.float32)
        for b in range(BATCH):
            for hl in range(HPC):
                d = slice(64 * hl, 64 * hl + 64)
                tok = slice(b * SEQ, (b + 1) * SEQ)
                qb = q[tok, d]   # (2048, 64)
                kb = k[tok, d]
                vb = v[tok, d]
                v_aug = np.concatenate([vb, np.ones((SEQ, 1), np.float32)], 1)
                ctx = np.zeros((65, SEQ), np.float32)
                for ch in chunks:
                    E = np.zeros((128, ch["W"]), np.float32)
                    for j, r, w, o in ch["sc_raw"]:
                        kk = slice(j * 128, j * 128 + 128)
                        qq = slice(64 * r, 64 * r + w)
                        s = kb[kk, :] @ qb[qq, :].T  # (128 keys, w queries)
                        E[:, o:o + w] = np.exp(SCALE * s)
                    for j, cl, o, w, eo in ch["av_raw"]:
                        # class selects which key half contributes
                        va = np.zeros((128, 65), np.float32)
                        if cl in (0, 2):
                            va[0:64] = v_aug[j * 128:j * 128 + 64]
                        if cl in (1, 2):
                            va[64:128] = v_aug[j * 128 + 64:j * 128 + 128]
                        ctx[:, o:o + w] += va.T @ E[:, eo:eo + w]
                ctx_n = ctx[:64, :] / ctx[64:65, :]
                ctx_all[d, tok] = ctx_n
        partial = np.asarray(wo, np.float32)[:, f] @ ctx_all  # (1024, T)
        out += partial.T
    out = out + np.asarray(bo, np.float32)
    return out.reshape(BATCH, SEQ, EMBED_DIM)


# ---------------------------------------------------------------------------
# Bass/Tile kernel (one core's program; SPMD across 8 cores)
# ---------------------------------------------------------------------------
def _trace_core_program():
    import concourse.bass as bass
    import concourse.mybir as mybir
    import concourse.tile as tile
    from concourse import bacc

    dt = mybir.dt
    chunks = build_schedule()

    nc = bacc.Bacc(None, target_bir_lowering=False)
    with tile.TileContext(nc) as tc:
        with ExitStack() as top:
            dram = top.enter_context(tc.tile_pool(name="dram", bufs=1, space="DRAM"))
            hT_d = dram.tile([EMBED_DIM, T], dt.bfloat16, kind="ExternalInput",
                             name="hT", uniquify=False)
            wqkT_d = dram.tile([EMBED_DIM, 2 * FPC], dt.bfloat16,
                               kind="ExternalInput", name="wqkT", uniquify=False)
            wvT_d = dram.tile([EMBED_DIM, FPC], dt.bfloat16,
                              kind="ExternalInput", name="wvT", uniquify=False)
            woT_d = dram.tile([FPC, EMBED_DIM], dt.bfloat16,
                              kind="ExternalInput", name="woT", uniquify=False)
            out_d = dram.tile([EMBED_DIM, T], dt.bfloat16,
                              kind="ExternalOutput", name="out", uniquify=False)

            # ---- persistent SBUF tensors -----------------------------------
            persist = top.enter_context(tc.tile_pool(name="persist", bufs=1))
            wqk = persist.tile([128, 8, 2 * FPC], dt.bfloat16, name="wqk_sb")
            wvT = persist.tile([128, 8, FPC], dt.bfloat16, name="wv_sb")
            woT = persist.tile([128, EMBED_DIM], dt.bfloat16, name="wo_sb")
            # q/k head-major on 64 partitions (base-0 only: matmuls with
            # base-partition-64 contraction operands hit a codegen/HW bug)
            q_sb = persist.tile([64, HPC * T], dt.bfloat16, name="q_sb")
            k_sb = persist.tile([64, HPC * T], dt.bfloat16, name="k_sb")
            # per (b, hl): zero-padded v^T slots, one per 64-key block m:
            # rows (m%2)*64..+64 hold [v | 1], the other 64 rows are zero,
            # so every AV matmul is K=128 at base partition 0.
            vaug = persist.tile([128, BATCH * HPC, NB * 128], dt.bfloat16,
                                name="vaug_sb")
            # interleaved (both blocks of a key tile) for full AV pieces
            vaug2 = persist.tile([128, BATCH * HPC, NKT * 128], dt.bfloat16,
                                 name="vaug2_sb")
            ctx_all = persist.tile([128, T], dt.bfloat16, name="ctx_sb")

            nc.sync.dma_start(out=wqk[:], in_=wqkT_d.rearrange(
                "(e p) f -> p e f", p=128))
            nc.sync.dma_start(out=wvT[:], in_=wvT_d.rearrange(
                "(e p) f -> p e f", p=128))
            nc.gpsimd.dma_start(out=woT[:], in_=woT_d[:])
            # zero halves + ones columns + pad of the 128-wide v slots
            # (disjoint from the data ranges the proj-phase copies write).
            # gpsimd takes half; its tensor-op library loads here, early,
            # and the partition-broadcast lib is loaded right after so no
            # swap lands mid-attention.
            for p in range(BATCH * HPC):
                slots = vaug[:, p, :].rearrange("p (m c) -> p m c", c=128)
                eng = nc.vector if p % 2 else nc.gpsimd
                eng.memset(slots[0:64, 1::2, :], 0.0)
                eng.memset(slots[64:128, 0::2, :], 0.0)
                eng.memset(slots[0:64, 0::2, 64:65], 1.0)
                eng.memset(slots[0:64, 0::2, 65:128], 0.0)
                eng.memset(slots[64:128, 1::2, 64:65], 1.0)
                eng.memset(slots[64:128, 1::2, 65:128], 0.0)
                s2 = vaug2[:, p, :].rearrange("p (m c) -> p m c", c=128)
                eng.memset(s2[:, :, 64:65], 1.0)
                eng.memset(s2[:, :, 65:128], 0.0)
            # preload the gpsimd partition-broadcast library (first use
            # otherwise costs a ~7us lib swap mid-attention)
            bc_warm = persist.tile([64, 16], dt.bfloat16, name="bc_warm")
            nc.gpsimd.partition_broadcast(bc_warm[:], wqk[0:1, 0, 0:16])

            NCHUNK = T // 512
            hp = top.enter_context(tc.tile_pool(name="hT_pool", bufs=1))
            hT = hp.tile([128, 8, T], dt.bfloat16, name="hT_sb")

            # ================================================================
            # The whole kernel is software-pipelined by batch:
            #   proj(b0) -> [attn pairs b0  ||  proj(b1) interleaved]
            #            -> [attn pairs b1  ||  out-proj(b0) interleaved]
            #            -> out-proj(b1)
            # The projection / out-proj matmuls are independent PE work that
            # fills the exp-coupled gaps of the attention pipeline, so the
            # PE never idles long enough for the HAM clock gate to throttle.
            # ================================================================

            def emit_proj_chunk(pps, vps, n):
                """Generator: one 512-token projection chunk, yielding
                between ~0.2-0.4us atoms of PE/copy work."""
                tsl = slice(512 * n, 512 * (n + 1))
                for which, (wsl, dst) in enumerate(
                        [(slice(0, 128), q_sb), (slice(128, 256), k_sb)]):
                    ps = pps.tile([128, 512], dt.float32, tag="proj")
                    for e in range(8):
                        nc.tensor.matmul(ps[:], wqk[:, e, wsl],
                                         hT[:, e, tsl],
                                         start=(e == 0), stop=(e == 7),
                                         skip_group_check=True)
                        if e % 2:
                            yield
                    for hl in range(HPC):
                        src = ps[64 * hl:64 * hl + 64, :]
                        d2 = dst[:, hl * T + 512 * n: hl * T + 512 * n + 512]
                        if (which + hl) % 2:
                            nc.scalar.copy(d2, src)
                        else:
                            nc.vector.tensor_copy(d2, src)
                        yield
                for m in range(4):
                    tb = 4 * n + m          # global 128-token block
                    b, jj = divmod(tb, NKT)
                    vp = vps.tile([128, FPC], dt.float32, tag="v")
                    for e in range(8):
                        nc.tensor.matmul(
                            vp[:], hT[:, e, 128 * tb:128 * tb + 128],
                            wvT[:, e, :], start=(e == 0), stop=(e == 7),
                            skip_group_check=True)
                        if e % 3 == 2:
                            yield
                    p0 = HPC * b
                    v3 = vp.rearrange("p (h c) -> p h c", c=64)
                    eng = (nc.vector.tensor_copy if m % 2 else nc.scalar.copy)
                    eng2 = (nc.scalar.copy if m % 2 else nc.vector.tensor_copy)
                    eng(vaug2[:, p0:p0 + 2, 128 * jj:128 * jj + 64], v3[:])
                    eng2(vaug[0:64, p0:p0 + 2,
                              128 * 2 * jj:128 * 2 * jj + 64], v3[0:64])
                    eng(vaug[64:128, p0:p0 + 2,
                             128 * (2 * jj + 1):128 * (2 * jj + 1) + 64],
                        v3[64:128])
                    yield

            def emit_proj_chunk_attn(pps, vps, n):
                """Like emit_proj_chunk but all copies on Vector (ACT is the
                exp bottleneck while attention runs)."""
                tsl = slice(512 * n, 512 * (n + 1))
                for which, (wsl, dst) in enumerate(
                        [(slice(0, 128), q_sb), (slice(128, 256), k_sb)]):
                    ps = pps.tile([128, 512], dt.float32, tag="proj")
                    for e in range(8):
                        nc.tensor.matmul(ps[:], wqk[:, e, wsl],
                                         hT[:, e, tsl],
                                         start=(e == 0), stop=(e == 7),
                                         skip_group_check=True)
                        if e % 2:
                            yield
                    for hl in range(HPC):
                        nc.vector.tensor_copy(
                            dst[:, hl * T + 512 * n: hl * T + 512 * n + 512],
                            ps[64 * hl:64 * hl + 64, :])
                        yield
                for m in range(4):
                    tb = 4 * n + m
                    b, jj = divmod(tb, NKT)
                    vp = vps.tile([128, FPC], dt.float32, tag="v")
                    for e in range(8):
                        nc.tensor.matmul(
                            vp[:], hT[:, e, 128 * tb:128 * tb + 128],
                            wvT[:, e, :], start=(e == 0), stop=(e == 7),
                            skip_group_check=True)
                        if e % 3 == 2:
                            yield
                    p0 = HPC * b
                    v3 = vp.rearrange("p (h c) -> p h c", c=64)
                    nc.vector.tensor_copy(
                        vaug2[:, p0:p0 + 2, 128 * jj:128 * jj + 64], v3[:])
                    yield
                    nc.vector.tensor_copy(
                        vaug[0:64, p0:p0 + 2,
                             128 * 2 * jj:128 * 2 * jj + 64], v3[0:64])
                    nc.vector.tensor_copy(
                        vaug[64:128, p0:p0 + 2,
                             128 * (2 * jj + 1):128 * (2 * jj + 1) + 64],
                        v3[64:128])
                    yield

            def emit_op_half(opp, opsb, half, vector_only):
                """Generator: out-proj for tokens half*2048..+2048 (batch
                `half`), 8 eo groups of (4 matmuls + 4 casts + 1 DMA)."""
                for eo in range(8):
                    ob = opsb.tile([128, T // 2], dt.bfloat16, tag="ob")
                    for nn in range(4):
                        n = 4 * half + nn
                        tsl = slice(512 * n, 512 * (n + 1))
                        ps = opp.tile([128, 512], dt.float32, tag="op")
                        nc.tensor.matmul(
                            ps[:], woT[:, 128 * eo:128 * eo + 128],
                            ctx_all[:, tsl], start=True, stop=True,
                            skip_group_check=True)
                        yield
                        if vector_only:
                            nc.vector.tensor_copy(
                                ob[:, 512 * nn:512 * nn + 512], ps[:])
                        else:
                            (nc.scalar.copy if nn % 2 else
                             nc.vector.tensor_copy)(
                                ob[:, 512 * nn:512 * nn + 512], ps[:])
                        yield
                    nc.sync.dma_start(
                        out=out_d[128 * eo:128 * eo + 128,
                                  (T // 2) * half:(T // 2) * (half + 1)],
                        in_=ob[:])
                    yield

            def emit_pair(scp, ctxp, ep, fp, b, hl, pump):
                """One (batch, head) attention pair; calls pump() between
                chunks/quarters to interleave independent PE work."""
                from concourse.ap import AP as RawAP
                p = b * HPC + hl
                qtok0 = hl * T + b * SEQ
                ctok0 = b * SEQ

                def pair_ap(base, stride, two, w):
                    """[P, 2, w] strided AP from a [P, w] slice."""
                    if not two:
                        return base
                    return RawAP(base.tensor, base.offset,
                                 [list(base.ap[0]), [stride, 2], [1, w]])

                E_tiles = []
                for ci, ch in enumerate(chunks):
                    W = ch["W"]
                    S = scp.tile([128, CHUNK_W], dt.float32, tag="S")
                    E = ep.tile([128, W], dt.bfloat16,
                                tag=f"E{ci}", name=f"E{ci}")
                    E_tiles.append(E)
                    for j, pieces in ch["sc"]:
                        kcol0 = qtok0 + 128 * j
                        lhsT = k_sb[:, kcol0:kcol0 + 128]
                        (r1, w, o1) = pieces[0]
                        two = len(pieces) == 2
                        rstride = 64 * (pieces[1][0] - r1) if two else 0
                        ostride = pieces[1][2] - o1 if two else 0
                        qc = qtok0 + 64 * r1
                        rhs = pair_ap(q_sb[:, qc:qc + w], rstride, two, w)
                        dst = pair_ap(S[:, o1:o1 + w], ostride, two, w)
                        nc.tensor.matmul(dst, lhsT, rhs,
                                         start=True, stop=True)
                    nc.scalar.activation(
                        E[:, :W], S[:, :W],
                        mybir.ActivationFunctionType.Exp, scale=SCALE)
                    pump()
                # AV accumulate (+Z via ones column) into four [65, 512]
                # quarter tiles; each quarter's 1/Z chain starts right
                # after its last AV matmul and never blocks the PE.
                for quarter in range(4):
                    lo = 512 * quarter
                    ctxq = ctxp.tile([128, 512], dt.float32, tag="ctxq")
                    started = False
                    for ci, ch in enumerate(chunks):
                        E = E_tiles[ci]
                        for j, c, pieces in ch["avq"][quarter]:
                            if c == 2:
                                lhsT = vaug2[:, p, 128 * j:128 * j + 128]
                            else:
                                m = 2 * j + c
                                lhsT = vaug[:, p, 128 * m:128 * m + 128]
                            (o1, w, e1) = pieces[0]
                            two = len(pieces) == 2
                            ostride = pieces[1][0] - o1 if two else 0
                            estride = pieces[1][2] - e1 if two else 0
                            rhs = pair_ap(E[:, e1:e1 + w], estride, two, w)
                            dst = pair_ap(ctxq[:, o1 - lo:o1 - lo + w],
                                          ostride, two, w)
                            nc.tensor.matmul(dst, lhsT, rhs,
                                             start=not started, stop=False,
                                             skip_group_check=True)
                            started = True
                    # finalize: 1/Z spread over 64 partitions via DMA
                    # reshape (single-lane reciprocal is ~8cyc/elem), then
                    # broadcast-multiply into ctx_all.
                    zrow = fp.tile([1, 512], dt.float32, tag="zrow")
                    nc.vector.tensor_copy(zrow[:], ctxq[64:65, :])
                    zsp = fp.tile([64, 8], dt.float32, tag="zsp")
                    nc.gpsimd.dma_start(out=zsp[:], in_=zrow[:])
                    rsp = fp.tile([64, 8], dt.float32, tag="rsp")
                    nc.vector.reciprocal(rsp[:], zsp[:])
                    rrow = fp.tile([1, 512], dt.float32, tag="rrow")
                    nc.gpsimd.dma_start(out=rrow[:], in_=rsp[:])
                    rbc = fp.tile([64, 512], dt.float32, tag="rbc")
                    nc.gpsimd.partition_broadcast(rbc[:], rrow[:])
                    nc.vector.tensor_tensor(
                        out=ctx_all[64 * hl:64 * hl + 64,
                                    ctok0 + lo:ctok0 + lo + 512],
                        in0=ctxq[0:64, :],
                        in1=rbc[:],
                        op=mybir.AluOpType.mult)
                    pump()
                    pump()

            # ---------------- emission ----------------
            # phase A: batch-0 projection alone (4 psum banks), then close
            # the pools so attention gets the banks.
            with tc.tile_pool(name="proj_ps0", bufs=2, space="PSUM") as pps0, \
                    tc.tile_pool(name="v_ps0", bufs=1, space="PSUM") as vps0:
                # ~3.5us of dummy matmuls while hT streams in: flips the
                # HAM clock gate to 8/8 before the projection starts
                warm = pps0.tile([128, 512], dt.float32, tag="warm", bufs=1)
                for i in range(16):
                    nc.tensor.matmul(warm[:, 0:256], wqk[:, 0, 0:128],
                                     wqk[:, 0, :], start=(i == 0),
                                     stop=(i == 15), skip_group_check=True)
                for n in range(NCHUNK):
                    for e in range(8):
                        nc.sync.dma_start(
                            out=hT[:, e, 512 * n:512 * n + 512],
                            in_=hT_d[128 * e:128 * e + 128,
                                     512 * n:512 * n + 512])
                for n in range(4):
                    for _ in emit_proj_chunk(pps0, vps0, n):
                        pass

            with tc.tile_pool(name="sc_ps", bufs=2, space="PSUM") as scp, \
                    tc.tile_pool(name="ctx_ps", bufs=2, space="PSUM") as ctxp, \
                    tc.tile_pool(name="e_pool", bufs=1) as ep, \
                    tc.tile_pool(name="fin_pool", bufs=2) as fp:
                # phase B: batch-0 attention with batch-1 projection
                # interleaved (single-buffer proj pools: 2 banks)
                with tc.tile_pool(name="proj_ps1", bufs=1,
                                  space="PSUM") as pps1, \
                        tc.tile_pool(name="v_ps1", bufs=1,
                                     space="PSUM") as vps1:
                    gen = (x for n in range(4, 8)
                           for x in emit_proj_chunk_attn(pps1, vps1, n))

                    def pump_proj():
                        for _ in range(2):
                            next(gen, None)

                    emit_pair(scp, ctxp, ep, fp, 0, 0, pump_proj)
                    emit_pair(scp, ctxp, ep, fp, 0, 1, pump_proj)
                    for _ in gen:   # drain leftovers
                        pass

                # phase C: batch-1 attention with batch-0 out-proj
                # interleaved (delayed so its first matmul never heads the
                # PE queue before ctx_all b0 is complete)
                with tc.tile_pool(name="op_ps", bufs=2, space="PSUM") as opp, \
                        tc.tile_pool(name="op_sb", bufs=1) as opsb:
                    ogen = iter(emit_op_half(opp, opsb, 0, True))
                    delay = [6]

                    def pump_op():
                        if delay[0] > 0:
                            delay[0] -= 1
                            return
                        for _ in range(3):
                            next(ogen, None)

                    emit_pair(scp, ctxp, ep, fp, 1, 0, pump_op)
                    emit_pair(scp, ctxp, ep, fp, 1, 1, pump_op)
                    for _ in ogen:
                        pass
                    # terminal out-proj: batch 1, with keep-warm dummy
                    # matmuls woven in (this stretch is cast-bound; idle PE
                    # re-throttles the clock and doubles the matmul time)
                    keeper = scp.tile([128, CHUNK_W], dt.float32,
                                      tag="S", name="keeper")
                    for _ in emit_op_half(opp, opsb, 1, False):
                        nc.tensor.matmul(
                            keeper[:, 0:512], wqk[:, 0, 0:128],
                            woT[:, 0:512], start=True, stop=True,
                            skip_group_check=True)

    nc.compile()
    return nc


_NC_CACHE = None


def make_in_maps(hs, wq, wk, wv, wo):
    hT = np.ascontiguousarray(
        np.asarray(hs, np.float32).reshape(T, EMBED_DIM).T).astype(BF16)
    wq = np.asarray(wq, np.float32)
    wk = np.asarray(wk, np.float32)
    wv = np.asarray(wv, np.float32)
    wo = np.asarray(wo, np.float32)
    in_maps = []
    for c in range(N_CORES):
        f = slice(FPC * c, FPC * (c + 1))
        wqkT = np.concatenate([wq[f, :].T, wk[f, :].T], axis=1)  # (1024, 256)
        in_maps.append({
            "hT": hT,
            "wqkT": np.ascontiguousarray(wqkT).astype(BF16),
            "wvT": np.ascontiguousarray(wv[f, :].T).astype(BF16),
            "woT": np.ascontiguousarray(wo[:, f].T).astype(BF16),
        })
    return in_maps


def kernel(hidden_states, wq, bq, wk, bk, wv, bv, wo, bo):
    global _NC_CACHE
    hs = np.asarray(hidden_states, np.float32)
    wq = np.asarray(wq, np.float32)
    wk = np.asarray(wk, np.float32)
    wv = np.asarray(wv, np.float32)
    wo = np.asarray(wo, np.float32)
    bq = np.asarray(bq, np.float32)
    bk = np.asarray(bk, np.float32)
    bv = np.asarray(bv, np.float32)
    bo = np.asarray(bo, np.float32)
    assert hs.shape == (BATCH, SEQ, EMBED_DIM)
    # biases bq/bk/bv are zero in this problem; fold nonzero ones on host
    # by shifting is impossible (they pass through nonlinearities), so
    # guard loudly rather than silently returning wrong results.
    for name, bias in (("bq", bq), ("bk", bk), ("bv", bv)):
        if np.abs(bias).max() != 0:
            raise NotImplementedError(f"nonzero {name} not supported")

    from concourse.bass_utils import run_bass_kernel_spmd

    if _NC_CACHE is None:
        _NC_CACHE = _trace_core_program()
    nc = _NC_CACHE

    in_maps = make_in_maps(hs, wq, wk, wv, wo)
    res = run_bass_kernel_spmd(nc, in_maps, list(range(N_CORES)))
    acc = np.zeros((EMBED_DIM, T), np.float32)
    for c in range(N_CORES):
        acc += res.results[c]["out"].astype(np.float32)
    out = acc.T + bo[None, :]
    return out.reshape(BATCH, SEQ, EMBED_DIM).astype(np.float32)


# revision 15
# speedup vs baseline: 1.0899x; 1.0899x over previous
"""BigBird block-sparse attention for Trainium2, 8-core SPMD.

Sharding: head-parallel. Each core owns 2 of the 16 heads (both batches).
  - q/k/v projections computed only for the core's 128 feature slice
    (full hidden_states replicated, weights sliced column-wise).
  - attention fully local per (batch, head).
  - out_proj tensor-parallel on the head (contraction) dim: each core
    emits a full-shape partial; the host sums the 8 partials (cheaper
    than a 16MB on-device all-reduce) and adds the output bias.

On-device layout choices (v2):
  - q/k feature-major (features on partitions, tokens on free dim);
    v projected TOKEN-major directly (hT chunk as the stationary matmul
    operand) so the zero-padded v^T slots are filled by plain strided
    copies -- no PE transposes, no transpose phase, PE never idles long
    enough for the HAM clock gate to re-throttle.
  - scores computed transposed: S_T[key, query] = k_j^T q, so that
    * AV is a natural matmul (contraction = keys = partitions),
    * the softmax denominator Z falls out of a ones-column appended to
      V^T,
    * normalization folds into the PSUM->SBUF context copy as a
      partition-broadcast multiply by 1/Z.
  - softmax skips max-subtraction (scores are O(1) after the 1/8 scale;
    exp cannot overflow fp32 for this distribution; softmax is shift
    invariant so the reference is matched).
  - BigBird mask is data independent and block-constant (64x64): it is
    evaluated at trace time into run-lists of attending query blocks per
    128-wide key tile.  No mask tensors on device at all.
  - NO filler matmuls: exp reads stale PSUM in the unwritten half-rows
    of half pieces; those E values are garbage but AV only ever reads
    the real seg ranges (the zero-padded v slots never touch them).
  - softmax normalization (1/Z) is finalized per 1024-column ctx half
    as soon as that half's AV matmuls are issued, so the reciprocal
    chain (copy -> DMA-spread -> recip -> DMA -> broadcast -> mult)
    hides under the other half / the next pair's score matmuls.
"""

import numpy as np
import ml_dtypes
from contextlib import ExitStack

# ----- problem constants (hardcoded per contract) --------------------------
EMBED_DIM = 1024
NUM_HEADS = 16
HEAD_DIM = 64           # d per head
WINDOW = 3
N_RAND = 3
BLOCK = 64
BATCH = 2
SEQ = 2048
NB = SEQ // BLOCK       # 32 key/query blocks per sequence
N_CORES = 8
HPC = NUM_HEADS // N_CORES      # heads per core = 2
FPC = HPC * HEAD_DIM            # feature slice per core = 128
T = BATCH * SEQ                 # 4096 tokens
NKT = NB // 2                   # 16 key tiles of 128 keys per (b,h)
SCALE = HEAD_DIM ** -0.5

BF16 = ml_dtypes.bfloat16

# score-chunk window width in psum columns (2 PSUM banks)
CHUNK_W = 1024
PSUM_BANK = 512  # fp32 elements per bank


def _block_attend() -> np.ndarray:
    """attend[r, kb]: query block r attends key block kb.

    Block-granular replica of the reference _bigbird_mask (the mask is
    block-constant: global first block rows/cols, +-WINDOW band, and
    N_RAND random blocks per row drawn with RandomState(0))."""
    att = np.zeros((NB, NB), dtype=bool)
    att[0, :] = True
    att[:, 0] = True
    blk = np.arange(NB)
    att |= np.abs(blk[:, None] - blk[None, :]) <= WINDOW
    rng = np.random.RandomState(0)
    for b in range(1, NB):
        avail = [x for x in range(1, NB) if abs(x - b) > WINDOW]
        if avail:
            sel = rng.choice(avail, size=min(N_RAND, len(avail)), replace=False)
            att[b, sel] = True
    return att


def _runs_of(mask_1d: np.ndarray):
    """[(r0, nblocks)] maximal runs of consecutive True entries."""
    runs = []
    for r in np.flatnonzero(mask_1d):
        if runs and runs[-1][0] + runs[-1][1] == r:
            runs[-1][1] += 1
        else:
            runs.append([int(r), 1])
    return [(r0, n) for r0, n in runs]


def build_schedule():
    """Global score-column layout + chunking + matmul pairing.

    Scores use ONLY full-tile (128-key) stationary operands over the
    UNION of attending query blocks per key tile: a matmul's cost is its
    streamed column count regardless of stationary width, and the
    zero-padded v slots already discard the non-attending half during
    AV, so splitting scores per 64-key half is pure overhead.

    The per-tile union layouts are concatenated into one global column
    space and cut into CHUNK_W psum windows (not tile-aligned).  AV
    pieces are the per-class (both / kb-even only / kb-odd only) column
    runs, split at chunk and ctx-quarter boundaries.

    Equal-width piece pairs (same key tile / class / psum bank) are
    merged into single matmuls via strided 3-D access patterns, halving
    the fixed per-matmul cost (~170ns SBUF latency + dispatch).

    Returns (chunks, av_q) where
      chunks: [{W, sc: [(j, [(r0, w, off)])], av ignored}]
      and each chunk's av mms, grouped by ctx quarter, are in
      chunk["avq"][quarter] = [(j, cls, [(out_o, w, e_off)])].
    """
    att = _block_attend()
    sc_pieces = []   # (j, r0, nblk, goff)
    av_pieces = []   # (j, cls, r0, nblk, goff)
    goff = 0
    for j in range(NKT):
        a0, a1 = att[:, 2 * j], att[:, 2 * j + 1]
        u = a0 | a1
        cls_arr = np.where(a0 & a1, 2, np.where(a0, 0, 1))
        for r0, n in _runs_of(u):
            sc_pieces.append((j, r0, n, goff))
            i = r0
            while i < r0 + n:
                c = int(cls_arr[i])
                k = i
                while k < r0 + n and cls_arr[k] == c:
                    k += 1
                av_pieces.append((j, c, i, k - i, goff + 64 * (i - r0)))
                i = k
            goff += 64 * n
    W_TOT = goff

    def pair_up(raw, keyfn):
        """Greedy pairing of equal-width pieces. raw items end with a
        sort position; returns list of (head..., [piece, piece?])."""
        from collections import defaultdict
        groups = defaultdict(list)
        order = []
        for it in raw:
            kk = keyfn(it)
            if kk not in groups:
                order.append(kk)
            groups[kk].append(it)
        out = []
        for kk in order:
            g = groups[kk]
            i = 0
            while i < len(g):
                if i + 1 < len(g):
                    out.append([g[i], g[i + 1]])
                    i += 2
                else:
                    out.append([g[i]])
                    i += 1
        return out

    chunks = []
    n_chunks = -(-W_TOT // CHUNK_W)
    for wi in range(n_chunks):
        lo, hi = wi * CHUNK_W, min(W_TOT, (wi + 1) * CHUNK_W)
        W = hi - lo
        # ---- scores: clip, split at S psum banks, pair ----
        sc_raw = []
        for j, r0, n, go in sc_pieces:
            s, e = go, go + 64 * n
            cs, ce = max(s, lo), min(e, hi)
            if cs >= ce:
                continue
            for o, w in _bank_split(cs - lo, ce - cs):
                r = r0 + (lo + o - s) // 64
                sc_raw.append((j, r, w, o))
        sc_pairs = pair_up(sc_raw,
                           lambda t: (t[0], t[2], t[3] // PSUM_BANK))
        sc_mms = []
        for pp in sc_pairs:
            j = pp[0][0]
            pieces = sorted([(r, w, o) for (_j, r, w, o) in pp],
                            key=lambda x: x[2])
            sc_mms.append((j, pieces))
        # ---- AV: clip, split at ctx quarters, pair, group by quarter --
        av_raw = []
        for j, c, r0, n, go in av_pieces:
            s, e = go, go + 64 * n
            cs, ce = max(s, lo), min(e, hi)
            if cs >= ce:
                continue
            r_lo = r0 + (cs - s) // 64
            for o, w in _bank_split(64 * r_lo, ce - cs):
                e_off = (cs - lo) + (o - 64 * r_lo)
                av_raw.append((j, c, o, w, e_off))
        avq = [[] for _ in range(4)]
        av_pairs = pair_up(av_raw,
                           lambda t: (t[0], t[1], t[3], t[2] // PSUM_BANK))
        for pp in av_pairs:
            j, c = pp[0][0], pp[0][1]
            pieces = sorted([(o, w, eo) for (_j, _c, o, w, eo) in pp],
                            key=lambda x: x[0])
            avq[pieces[0][0] // PSUM_BANK].append((j, c, pieces))
        chunks.append(dict(W=W, sc=sc_mms, avq=avq,
                           sc_raw=sc_raw, av_raw=av_raw))
    return chunks


def _bank_split(off, w, bank=PSUM_BANK):
    """split [off, off+w) at bank boundaries -> [(off, w), ...]"""
    out = []
    while w > 0:
        room = bank - (off % bank)
        take = min(room, w)
        out.append((off, take))
        off += take
        w -= take
    return out


# ---------------------------------------------------------------------------
# numpy golden of the exact on-device algorithm (fp32, validates schedule)
# ---------------------------------------------------------------------------
def numpy_golden(hidden_states, wq, bq, wk, bk, wv, bv, wo, bo):
    hs = np.asarray(hidden_states, np.float32).reshape(T, EMBED_DIM)
    chunks = build_schedule()
    out = np.zeros((T, EMBED_DIM), np.float32)
    for c in range(N_CORES):
        f = slice(FPC * c, FPC * (c + 1))
        q = hs @ np.asarray(wq, np.float32)[f, :].T  # (T, 128)
        k = hs @ np.asarray(wk, np.float32)[f, :].T
        v = hs @ np.asarray(wv, np.float32)[f, :].T
        ctx_all = np.zeros((FPC, T), np.float32)
        for b in range(BATCH):
            for hl in range(HPC):
                d = slice(64 * hl, 64 * hl + 64)
                tok = slice(b * SEQ, (b + 1) * SEQ)
                qb = q[tok, d]   # (2048, 64)
                kb = k[tok, d]
                vb = v[tok, d]
                v_aug = np.concatenate([vb, np.ones((SEQ, 1), np.float32)], 1)
                ctx = np.zeros((65, SEQ), np.float32)
                for ch in chunks:
                    E = np.zeros((128, ch["W"]), np.float32)
                    for j, r, w, o in ch["sc_raw"]:
                        kk = slice(j * 128, j * 128 + 128)
                        qq = slice(64 * r, 64 * r + w)
                        s = kb[kk, :] @ qb[qq, :].T  # (128 keys, w queries)
                        E[:, o:o + w] = np.exp(SCALE * s)
                    for j, cl, o, w, eo in ch["av_raw"]:
                        # class selects which key half contributes
                        va = np.zeros((128, 65), np.float32)
                        if cl in (0, 2):
                            va[0:64] = v_aug[j * 128:j * 128 + 64]
                        if cl in (1, 2):
                            va[64:128] = v_aug[j * 128 + 64:j * 128 + 128]
                        ctx[:, o:o + w] += va.T @ E[:, eo:eo + w]
                ctx_n = ctx[:64, :] / ctx[64:65, :]
                ctx_all[d, tok] = ctx_n
        partial = np.asarray(wo, np.float32)[:, f] @ ctx_all  # (1024, T)
        out += partial.T
    out = out + np.asarray(bo, np.float32)
    return out.reshape(BATCH, SEQ, EMBED_DIM)


# ---------------------------------------------------------------------------
# Bass/Tile kernel (one core's program; SPMD across 8 cores)
# ---------------------------------------------------------------------------
def _trace_core_program():
    import concourse.bass as bass
    import concourse.mybir as mybir
    import concourse.tile as tile
    from concourse import bacc

    dt = mybir.dt
    chunks = build_schedule()

    nc = bacc.Bacc(None, target_bir_lowering=False)
    with tile.TileContext(nc) as tc:
        with ExitStack() as top:
            dram = top.enter_context(tc.tile_pool(name="dram", bufs=1, space="DRAM"))
            hT_d = dram.tile([EMBED_DIM, T], dt.bfloat16, kind="ExternalInput",
                             name="hT", uniquify=False)
            wqkT_d = dram.tile([EMBED_DIM, 2 * FPC], dt.bfloat16,
                               kind="ExternalInput", name="wqkT", uniquify=False)
            wvT_d = dram.tile([EMBED_DIM, FPC], dt.bfloat16,
                              kind="ExternalInput", name="wvT", uniquify=False)
            woT_d = dram.tile([FPC, EMBED_DIM], dt.bfloat16,
                              kind="ExternalInput", name="woT", uniquify=False)
            out_d = dram.tile([EMBED_DIM, T], dt.bfloat16,
                              kind="ExternalOutput", name="out", uniquify=False)

            # ---- persistent SBUF tensors -----------------------------------
            persist = top.enter_context(tc.tile_pool(name="persist", bufs=1))
            wqk = persist.tile([128, 8, 2 * FPC], dt.bfloat16, name="wqk_sb")
            wvT = persist.tile([128, 8, FPC], dt.bfloat16, name="wv_sb")
            woT = persist.tile([128, EMBED_DIM], dt.bfloat16, name="wo_sb")
            # q/k head-major on 64 partitions (base-0 only: matmuls with
            # base-partition-64 contraction operands hit a codegen/HW bug)
            q_sb = persist.tile([64, HPC * T], dt.bfloat16, name="q_sb")
            k_sb = persist.tile([64, HPC * T], dt.bfloat16, name="k_sb")
            # per (b, hl): zero-padded v^T slots, one per 64-key block m:
            # rows (m%2)*64..+64 hold [v | 1], the other 64 rows are zero,
            # so every AV matmul is K=128 at base partition 0.
            vaug = persist.tile([128, BATCH * HPC, NB * 128], dt.bfloat16,
                                name="vaug_sb")
            # interleaved (both blocks of a key tile) for full AV pieces
            vaug2 = persist.tile([128, BATCH * HPC, NKT * 128], dt.bfloat16,
                                 name="vaug2_sb")
            ctx_all = persist.tile([128, T], dt.bfloat16, name="ctx_sb")

            nc.sync.dma_start(out=wqk[:], in_=wqkT_d.rearrange(
                "(e p) f -> p e f", p=128))
            nc.sync.dma_start(out=wvT[:], in_=wvT_d.rearrange(
                "(e p) f -> p e f", p=128))
            nc.gpsimd.dma_start(out=woT[:], in_=woT_d[:])
            # zero halves + ones columns + pad of the 128-wide v slots
            # (disjoint from the data ranges the proj-phase copies write).
            # gpsimd takes half; its tensor-op library loads here, early,
            # and the partition-broadcast lib is loaded right after so no
            # swap lands mid-attention.
            for p in range(BATCH * HPC):
                slots = vaug[:, p, :].rearrange("p (m c) -> p m c", c=128)
                eng = nc.vector if p % 2 else nc.gpsimd
                eng.memset(slots[0:64, 1::2, 0:65], 0.0)
                eng.memset(slots[64:128, 0::2, 0:65], 0.0)
                eng.memset(slots[0:64, 0::2, 64:65], 1.0)
                eng.memset(slots[64:128, 1::2, 64:65], 1.0)
                s2 = vaug2[:, p, :].rearrange("p (m c) -> p m c", c=128)
                eng.memset(s2[:, :, 64:65], 1.0)
                # slot columns 65:128 stay uninitialized on purpose: as
                # stationary-operand columns they only produce ctx rows
                # 65:127, which nothing ever reads
            # preload the gpsimd partition-broadcast library (first use
            # otherwise costs a ~7us lib swap mid-attention)
            bc_warm = persist.tile([64, 16], dt.bfloat16, name="bc_warm")
            nc.gpsimd.partition_broadcast(bc_warm[:], wqk[0:1, 0, 0:16])

            NCHUNK = T // 512
            hp = top.enter_context(tc.tile_pool(name="hT_pool", bufs=1))
            hT = hp.tile([128, 8, T], dt.bfloat16, name="hT_sb")

            # ================================================================
            # The whole kernel is software-pipelined by batch:
            #   proj(b0) -> [attn pairs b0  ||  proj(b1) interleaved]
            #            -> [attn pairs b1  ||  out-proj(b0) interleaved]
            #            -> out-proj(b1)
            # The projection / out-proj matmuls are independent PE work that
            # fills the exp-coupled gaps of the attention pipeline, so the
            # PE never idles long enough for the HAM clock gate to throttle.
            # ================================================================

            def emit_proj_chunk(pps, vps, n):
                """Generator: one 512-token projection chunk, yielding
                between ~0.2-0.4us atoms of PE/copy work."""
                tsl = slice(512 * n, 512 * (n + 1))
                for which, (wsl, dst) in enumerate(
                        [(slice(0, 128), q_sb), (slice(128, 256), k_sb)]):
                    ps = pps.tile([128, 512], dt.float32, tag="proj")
                    for e in range(8):
                        nc.tensor.matmul(ps[:], wqk[:, e, wsl],
                                         hT[:, e, tsl],
                                         start=(e == 0), stop=(e == 7),
                                         skip_group_check=True)
                        if e % 2:
                            yield
                    for hl in range(HPC):
                        src = ps[64 * hl:64 * hl + 64, :]
                        d2 = dst[:, hl * T + 512 * n: hl * T + 512 * n + 512]
                        if (which + hl) % 2:
                            nc.scalar.copy(d2, src)
                        else:
                            nc.vector.tensor_copy(d2, src)
                        yield
                for m in range(4):
                    tb = 4 * n + m          # global 128-token block
                    b, jj = divmod(tb, NKT)
                    vp = vps.tile([128, FPC], dt.float32, tag="v")
                    for e in range(8):
                        nc.tensor.matmul(
                            vp[:], hT[:, e, 128 * tb:128 * tb + 128],
                            wvT[:, e, :], start=(e == 0), stop=(e == 7),
                            skip_group_check=True)
                        if e % 3 == 2:
                            yield
                    p0 = HPC * b
                    v3 = vp.rearrange("p (h c) -> p h c", c=64)
                    eng = (nc.vector.tensor_copy if m % 2 else nc.scalar.copy)
                    eng2 = (nc.scalar.copy if m % 2 else nc.vector.tensor_copy)
                    eng(vaug2[:, p0:p0 + 2, 128 * jj:128 * jj + 64], v3[:])
                    eng2(vaug[0:64, p0:p0 + 2,
                              128 * 2 * jj:128 * 2 * jj + 64], v3[0:64])
                    eng(vaug[64:128, p0:p0 + 2,
                             128 * (2 * jj + 1):128 * (2 * jj + 1) + 64],
                        v3[64:128])
                    yield

            def emit_proj_chunk_attn(pps, vps, n):
                """Like emit_proj_chunk but all copies on Vector (ACT is the
                exp bottleneck while attention runs)."""
                tsl = slice(512 * n, 512 * (n + 1))
                for which, (wsl, dst) in enumerate(
                        [(slice(0, 128), q_sb), (slice(128, 256), k_sb)]):
                    ps = pps.tile([128, 512], dt.float32, tag="proj")
                    for e in range(8):
                        nc.tensor.matmul(ps[:], wqk[:, e, wsl],
                                         hT[:, e, tsl],
                                         start=(e == 0), stop=(e == 7),
                                         skip_group_check=True)
                        if e % 2:
                            yield
                    for hl in range(HPC):
                        nc.vector.tensor_copy(
                            dst[:, hl * T + 512 * n: hl * T + 512 * n + 512],
                            ps[64 * hl:64 * hl + 64, :])
                        yield
                for m in range(4):
                    tb = 4 * n + m
                    b, jj = divmod(tb, NKT)
                    vp = vps.tile([128, FPC], dt.float32, tag="v")
                    for e in range(8):
                        nc.tensor.matmul(
                            vp[:], hT[:, e, 128 * tb:128 * tb + 128],
                            wvT[:, e, :], start=(e == 0), stop=(e == 7),
                            skip_group_check=True)
                        if e % 3 == 2:
                            yield
                    p0 = HPC * b
                    v3 = vp.rearrange("p (h c) -> p h c", c=64)
                    nc.vector.tensor_copy(
                        vaug2[:, p0:p0 + 2, 128 * jj:128 * jj + 64], v3[:])
                    yield
                    nc.vector.tensor_copy(
                        vaug[0:64, p0:p0 + 2,
                             128 * 2 * jj:128 * 2 * jj + 64], v3[0:64])
                    nc.vector.tensor_copy(
                        vaug[64:128, p0:p0 + 2,
                             128 * (2 * jj + 1):128 * (2 * jj + 1) + 64],
                        v3[64:128])
                    yield

            def emit_op_half(opp, opsb, half, vector_only):
                """Generator: out-proj for tokens half*2048..+2048 (batch
                `half`); double-buffered [128, 1024] staging tiles so the
                output DMA never serializes the next casts."""
                for eo in range(8):
                    for sub in range(2):
                        ob = opsb.tile([128, 1024], dt.bfloat16, tag="ob")
                        for nn in range(2):
                            n = 4 * half + 2 * sub + nn
                            tsl = slice(512 * n, 512 * (n + 1))
                            ps = opp.tile([128, 512], dt.float32, tag="op")
                            nc.tensor.matmul(
                                ps[:], woT[:, 128 * eo:128 * eo + 128],
                                ctx_all[:, tsl], start=True, stop=True,
                                skip_group_check=True)
                            yield
                            if vector_only:
                                nc.vector.tensor_copy(
                                    ob[:, 512 * nn:512 * nn + 512], ps[:])
                            else:
                                (nc.scalar.copy if nn % 2 else
                                 nc.vector.tensor_copy)(
                                    ob[:, 512 * nn:512 * nn + 512], ps[:])
                            yield
                        nc.sync.dma_start(
                            out=out_d[128 * eo:128 * eo + 128,
                                      (T // 2) * half + 1024 * sub:
                                      (T // 2) * half + 1024 * sub + 1024],
                            in_=ob[:])
                        yield

            def emit_pair(scp, ctxp, ep, fp, b, hl, pump):
                """One (batch, head) attention pair; calls pump() between
                chunks/quarters to interleave independent PE work."""
                from concourse.ap import AP as RawAP
                p = b * HPC + hl
                qtok0 = hl * T + b * SEQ
                ctok0 = b * SEQ

                def pair_ap(base, stride, two, w):
                    """[P, 2, w] strided AP from a [P, w] slice."""
                    if not two:
                        return base
                    return RawAP(base.tensor, base.offset,
                                 [list(base.ap[0]), [stride, 2], [1, w]])

                E_tiles = []
                for ci, ch in enumerate(chunks):
                    W = ch["W"]
                    S = scp.tile([128, CHUNK_W], dt.float32, tag="S")
                    E = ep.tile([128, W], dt.bfloat16,
                                tag=f"E{ci}", name=f"E{ci}")
                    E_tiles.append(E)
                    for j, pieces in ch["sc"]:
                        kcol0 = qtok0 + 128 * j
                        lhsT = k_sb[:, kcol0:kcol0 + 128]
                        (r1, w, o1) = pieces[0]
                        two = len(pieces) == 2
                        rstride = 64 * (pieces[1][0] - r1) if two else 0
                        ostride = pieces[1][2] - o1 if two else 0
                        qc = qtok0 + 64 * r1
                        rhs = pair_ap(q_sb[:, qc:qc + w], rstride, two, w)
                        dst = pair_ap(S[:, o1:o1 + w], ostride, two, w)
                        nc.tensor.matmul(dst, lhsT, rhs,
                                         start=True, stop=True)
                    nc.scalar.activation(
                        E[:, :W], S[:, :W],
                        mybir.ActivationFunctionType.Exp, scale=SCALE)
                    pump()
                # AV accumulate (+Z via ones column) into four [65, 512]
                # quarter tiles; each quarter's 1/Z chain starts right
                # after its last AV matmul and never blocks the PE.
                for quarter in range(4):
                    lo = 512 * quarter
                    ctxq = ctxp.tile([128, 512], dt.float32, tag="ctxq")
                    started = False
                    for ci, ch in enumerate(chunks):
                        E = E_tiles[ci]
                        for j, c, pieces in ch["avq"][quarter]:
                            if c == 2:
                                lhsT = vaug2[:, p, 128 * j:128 * j + 128]
                            else:
                                m = 2 * j + c
                                lhsT = vaug[:, p, 128 * m:128 * m + 128]
                            (o1, w, e1) = pieces[0]
                            two = len(pieces) == 2
                            ostride = pieces[1][0] - o1 if two else 0
                            estride = pieces[1][2] - e1 if two else 0
                            rhs = pair_ap(E[:, e1:e1 + w], estride, two, w)
                            dst = pair_ap(ctxq[:, o1 - lo:o1 - lo + w],
                                          ostride, two, w)
                            nc.tensor.matmul(dst, lhsT, rhs,
                                             start=not started, stop=False,
                                             skip_group_check=True)
                            started = True
                    # finalize: 1/Z spread over 64 partitions via DMA
                    # reshape (single-lane reciprocal is ~8cyc/elem), then
                    # broadcast-multiply into ctx_all.
                    zrow = fp.tile([1, 512], dt.float32, tag="zrow")
                    nc.vector.tensor_copy(zrow[:], ctxq[64:65, :])
                    zsp = fp.tile([64, 8], dt.float32, tag="zsp")
                    nc.gpsimd.dma_start(out=zsp[:], in_=zrow[:])
                    rsp = fp.tile([64, 8], dt.float32, tag="rsp")
                    nc.vector.reciprocal(rsp[:], zsp[:])
                    rrow = fp.tile([1, 512], dt.float32, tag="rrow")
                    nc.gpsimd.dma_start(out=rrow[:], in_=rsp[:])
                    rbc = fp.tile([64, 512], dt.float32, tag="rbc")
                    nc.gpsimd.partition_broadcast(rbc[:], rrow[:])
                    nc.vector.tensor_tensor(
                        out=ctx_all[64 * hl:64 * hl + 64,
                                    ctok0 + lo:ctok0 + lo + 512],
                        in0=ctxq[0:64, :],
                        in1=rbc[:],
                        op=mybir.AluOpType.mult)
                    pump()
                    pump()

            # ---------------- emission ----------------
            # phase A: batch-0 projection alone (4 psum banks), then close
            # the pools so attention gets the banks.
            with tc.tile_pool(name="proj_ps0", bufs=2, space="PSUM") as pps0, \
                    tc.tile_pool(name="v_ps0", bufs=1, space="PSUM") as vps0:
                # ~3.5us of dummy matmuls while hT streams in: flips the
                # HAM clock gate to 8/8 before the projection starts
                warm = pps0.tile([128, 512], dt.float32, tag="warm", bufs=1)
                for i in range(16):
                    nc.tensor.matmul(warm[:, 0:256], wqk[:, 0, 0:128],
                                     wqk[:, 0, :], start=(i == 0),
                                     stop=(i == 15), skip_group_check=True)
                for n in range(NCHUNK):
                    for e in range(8):
                        nc.sync.dma_start(
                            out=hT[:, e, 512 * n:512 * n + 512],
                            in_=hT_d[128 * e:128 * e + 128,
                                     512 * n:512 * n + 512])
                for n in range(4):
                    for _ in emit_proj_chunk(pps0, vps0, n):
                        pass

            with tc.tile_pool(name="sc_ps", bufs=2, space="PSUM") as scp, \
                    tc.tile_pool(name="ctx_ps", bufs=2, space="PSUM") as ctxp, \
                    tc.tile_pool(name="e_pool", bufs=1) as ep, \
                    tc.tile_pool(name="fin_pool", bufs=2) as fp:
                # phase B: batch-0 attention with batch-1 projection
                # interleaved (single-buffer proj pools: 2 banks)
                with tc.tile_pool(name="proj_ps1", bufs=1,
                                  space="PSUM") as pps1, \
                        tc.tile_pool(name="v_ps1", bufs=1,
                                     space="PSUM") as vps1:
                    gen = (x for n in range(4, 8)
                           for x in emit_proj_chunk_attn(pps1, vps1, n))

                    def pump_proj():
                        for _ in range(2):
                            next(gen, None)

                    emit_pair(scp, ctxp, ep, fp, 0, 0, pump_proj)
                    emit_pair(scp, ctxp, ep, fp, 0, 1, pump_proj)
                    for _ in gen:   # drain leftovers
                        pass

                # phase C: batch-1 attention with batch-0 out-proj
                # interleaved (delayed so its first matmul never heads the
                # PE queue before ctx_all b0 is complete)
                with tc.tile_pool(name="op_ps", bufs=2, space="PSUM") as opp, \
                        tc.tile_pool(name="op_sb", bufs=2) as opsb:
                    ogen = iter(emit_op_half(opp, opsb, 0, True))
                    delay = [6]

                    def pump_op():
                        if delay[0] > 0:
                            delay[0] -= 1
                            return
                        for _ in range(3):
                            next(ogen, None)

                    emit_pair(scp, ctxp, ep, fp, 1, 0, pump_op)
                    emit_pair(scp, ctxp, ep, fp, 1, 1, pump_op)
                    for _ in ogen:
                        pass
                    # terminal out-proj: batch 1, with keep-warm dummy
                    # matmuls woven in (this stretch is cast-bound; idle PE
                    # re-throttles the clock and doubles the matmul time)
                    keeper = scp.tile([128, CHUNK_W], dt.float32,
                                      tag="S", name="keeper")
                    for _ in emit_op_half(opp, opsb, 1, False):
                        nc.tensor.matmul(
                            keeper[:, 0:512], wqk[:, 0, 0:128],
                            woT[:, 0:512], start=True, stop=True,
                            skip_group_check=True)

    nc.compile()
    return nc


_NC_CACHE = None


def make_in_maps(hs, wq, wk, wv, wo):
    hT = np.ascontiguousarray(
        np.asarray(hs, np.float32).reshape(T, EMBED_DIM).T).astype(BF16)
    wq = np.asarray(wq, np.float32)
    wk = np.asarray(wk, np.float32)
    wv = np.asarray(wv, np.float32)
    wo = np.asarray(wo, np.float32)
    in_maps = []
    for c in range(N_CORES):
        f = slice(FPC * c, FPC * (c + 1))
        wqkT = np.concatenate([wq[f, :].T, wk[f, :].T], axis=1)  # (1024, 256)
        in_maps.append({
            "hT": hT,
            "wqkT": np.ascontiguousarray(wqkT).astype(BF16),
            "wvT": np.ascontiguousarray(wv[f, :].T).astype(BF16),
            "woT": np.ascontiguousarray(wo[:, f].T).astype(BF16),
        })
    return in_maps


def kernel(hidden_states, wq, bq, wk, bk, wv, bv, wo, bo):
    global _NC_CACHE
    hs = np.asarray(hidden_states, np.float32)
    wq = np.asarray(wq, np.float32)
    wk = np.asarray(wk, np.float32)
    wv = np.asarray(wv, np.float32)
    wo = np.asarray(wo, np.float32)
    bq = np.asarray(bq, np.float32)
    bk = np.asarray(bk, np.float32)
    bv = np.asarray(bv, np.float32)
    bo = np.asarray(bo, np.float32)
    assert hs.shape == (BATCH, SEQ, EMBED_DIM)
    # biases bq/bk/bv are zero in this problem; fold nonzero ones on host
    # by shifting is impossible (they pass through nonlinearities), so
    # guard loudly rather than silently returning wrong results.
    for name, bias in (("bq", bq), ("bk", bk), ("bv", bv)):
        if np.abs(bias).max() != 0:
            raise NotImplementedError(f"nonzero {name} not supported")

    from concourse.bass_utils import run_bass_kernel_spmd

    if _NC_CACHE is None:
        _NC_CACHE = _trace_core_program()
    nc = _NC_CACHE

    in_maps = make_in_maps(hs, wq, wk, wv, wo)
    res = run_bass_kernel_spmd(nc, in_maps, list(range(N_CORES)))
    acc = np.zeros((EMBED_DIM, T), np.float32)
    for c in range(N_CORES):
        acc += res.results[c]["out"].astype(np.float32)
    out = acc.T + bo[None, :]
    return out.reshape(BATCH, SEQ, EMBED_DIM).astype(np.float32)
